# revision 18
# baseline (speedup 1.0000x reference)
"""Trainium2 Bass kernel for nn_GAT_FP (3-layer GAT message passing), 8 cores.

Sharding: nodes split 1250/core (dst-owner), edges sorted by dst into 10
windows of 128 dst rows. Per window one dma_gather (queue-rotated across 4
SWDGE queues) pulls a merged per-source table row [fs|pfs|es|z] (1152 bf16);
dst-side expansion runs as selection-matrix matmuls on the PE with the
selection matrices built on the host and uploaded. GATv2 logits use the
leaky_relu split a.lrelu(s) = 0.6(a.s) + 0.4(a.|s|): the 0.6 term is folded
into the dense weights per node (pfs/pfd cols), the |s| term is Act-abs +
bf16 mult/reduce. Edge-softmax denominators ride as extra rhs columns of the
aggregation matmuls. Layer-1 dense (D1) is fused per window into the E0
epilogue; its gather table is 256B rows. Wide dense math is bf16 with fp32
PSUM accumulation.
"""
import sys
sys.path.insert(0, "/opt/trn_rl_repo")
import math
import numpy as np
import ml_dtypes

import concourse.bass as bass
import concourse.tile as tile
from concourse import bacc, mybir
from concourse.bass_utils import run_bass_kernel_spmd
from concourse.masks import make_identity

F32 = mybir.dt.float32
BF16 = mybir.dt.bfloat16
I16 = mybir.dt.int16
AF = mybir.ActivationFunctionType
OP = mybir.AluOpType
AX = mybir.AxisListType
NPBF = ml_dtypes.bfloat16

N, E, IN = 10000, 64000, 1247
H, D0, D1, OUT = 4, 256, 8, 6
HD0, HD1 = H * D0, H * D1            # 1024, 32
NC = 8
NPC = N // NC                        # 1250 nodes per core
WPC = (NPC + 127) // 128             # 10 windows per core
KA = IN + 1                          # 1248 augmented contraction dim
K0T = (KA + 127) // 128              # 10 k-tiles layer-0 dense
K1T = HD0 // 128                     # 8 k-tiles layer-1 dense
NEG = -30000.0                       # pad logit bias -> exp == 0
TW = 1152     # tab0 row (bf16): [fs|pfs|es|z 0:1064, pad]; 256B-mult rows
FDW = 1032    # fd side: [fd 0:1024 | pfd :1028 | ed :1032]
TW1 = 128     # tab1 row (bf16): [fs1 0:32 | pfs1 :36 | pad]; 256B-mult rows
FD1W = 36     # fd1 side: [fd1 0:32 | pfd1 :36]
NQ = 4        # SWDGE queues for gather rotation

_compiled = {}
last_exec_ns = None
last_result = None


def _wrows(w):
    return min(128, NPC - w * 128)


def _build_program(Ts):
    totT = sum(Ts)
    Tmax = max(Ts)
    toff = [sum(Ts[:w]) for w in range(WPC)]
    nc = bacc.Bacc("TRN2", target_bir_lowering=False, debug=False,
                   num_devices=NC, num_swdge_queues=NQ,
                   dynamic_dma_scratch_size=32768)

    feat = nc.dram_tensor("feat", [NPC, IN], F32, kind="ExternalInput")
    wl0a = nc.dram_tensor("wl0a", [KA, 1028], BF16, kind="ExternalInput")
    wr0a = nc.dram_tensor("wr0a", [KA, FDW], BF16, kind="ExternalInput")
    wres0a = nc.dram_tensor("wres0a", [KA, HD0], BF16, kind="ExternalInput")
    w2a = nc.dram_tensor("w2a", [KA, 36], BF16, kind="ExternalInput")
    wcat1 = nc.dram_tensor("wcat1", [HD0, 104], BF16, kind="ExternalInput")
    wlin = nc.dram_tensor("wlin", [2 * HD1, OUT], BF16, kind="ExternalInput")
    a0b04 = nc.dram_tensor("a0b04", [128, HD0], BF16, kind="ExternalInput")
    a1b04 = nc.dram_tensor("a1b04", [128, HD1], F32, kind="ExternalInput")
    b1bc = nc.dram_tensor("b1bc", [128, HD1], F32, kind="ExternalInput")
    blinbc = nc.dram_tensor("blinbc", [128, OUT], F32, kind="ExternalInput")
    srcidx = nc.dram_tensor("srcidx", [128, 8 * totT], I16, kind="ExternalInput")
    selb = nc.dram_tensor("selb", [128, totT * 128], BF16, kind="ExternalInput")
    selTb = nc.dram_tensor("selTb", [128, totT * 128], BF16,
                           kind="ExternalInput")
    vbias = nc.dram_tensor("vbias", [128, totT], F32, kind="ExternalInput")
    vbias4 = nc.dram_tensor("vbias4", [128, totT * 4], F32,
                            kind="ExternalInput")
    a1bt = nc.dram_tensor("a1bt", [128, Tmax * HD1], BF16,
                          kind="ExternalInput")
    out_ext = nc.dram_tensor("out", [NPC, OUT], F32, kind="ExternalOutput")

    with tile.TileContext(nc) as tc:
        with tc.tile_pool(name="dram", bufs=1, space="DRAM") as dram, \
             tc.tile_pool(name="constp", bufs=1) as constp, \
             tc.tile_pool(name="hold", bufs=1) as hold, \
             tc.tile_pool(name="work", bufs=2) as work:

            tab0_c = dram.tile([NPC, TW], BF16)
            tab1_c = dram.tile([NPC, TW1], BF16)
            tab0_full = dram.tile([N, TW], BF16, addr_space="Shared")
            tab1_full = dram.tile([N, TW1], BF16, addr_space="Shared")
            cs_bounce = dram.tile([1, IN], F32)
            cs_sum = dram.tile([1, IN], F32, addr_space="Shared")

            ident = constp.tile([128, 128], F32)
            make_identity(nc, ident[:])
            ident16 = constp.tile([128, 128], BF16)
            nc.vector.tensor_copy(out=ident16[:], in_=ident[:])
            ones128 = constp.tile([128, 1], F32)
            nc.vector.memset(ones128[:], 1.0)
            ones_row = constp.tile([1, 128], F32)
            nc.vector.memset(ones_row[:], 1.0)

            def load_const(name, dramt, shape, dt=F32):
                t = constp.tile(shape, dt, tag=name, name=name)
                nc.sync.dma_start(out=t[:], in_=dramt[:])
                return t
            a0b = load_const("a0b", a0b04, [128, HD0], BF16)
            a1b = load_const("a1b", a1b04, [128, HD1])
            b1b = load_const("b1b", b1bc, [128, HD1])
            blinb = load_const("blinb", blinbc, [128, OUT])
            vbs = load_const("vbs", vbias, [128, totT])
            vb4 = load_const("vb4", vbias4, [128, totT * 4])
            a1bT = load_const("a1bT", a1bt, [128, Tmax * HD1], BF16)
            sidx = load_const("sidx", srcidx, [128, 8 * totT], I16)
            wlsb = load_const("wlsb", wlin, [2 * HD1, OUT], BF16)

            dma_sems = [nc.alloc_semaphore(f"gsem{q}") for q in range(NQ)]
            for q in range(NQ):
                nc.gpsimd.sem_clear(dma_sems[q])
            qcount = [0] * NQ

            def gather_async(out_ap, table, co_t, ntiles, elem, queue):
                nc.gpsimd.dma_gather(
                    out_ap=out_ap, in_ap=table,
                    idxs_ap=sidx[:, 8 * co_t:8 * (co_t + ntiles)],
                    num_idxs=ntiles * 128, num_idxs_reg=ntiles * 128,
                    elem_size=elem, queue_num=queue,
                    prepare_only=True, sem=dma_sems[queue])
                nc.gpsimd.trigger_dma(count=None, queue_num=queue)
                qcount[queue] += 1
                return (queue, qcount[queue] * 16)

            def gather_wait(tokens, engines):
                for q, v in tokens:
                    for e in engines:
                        e.wait_ge(dma_sems[q], v)

            h3_all = hold.tile([128, WPC * HD1], BF16)
            fdw_all = hold.tile([128, WPC * FDW], BF16)
            res_all = hold.tile([128, WPC * HD0], BF16)
            fd1_all = hold.tile([128, WPC * FD1W], BF16)
            res1_t = [hold.tile([128, HD1], F32, tag=f"res1_{m}",
                                name=f"res1_{m}") for m in range(WPC)]

            # ============ Phase P + D0 (hT alive) ============
            with tc.tile_pool(name="hpool", bufs=1) as hpool:
                hT = hpool.tile([128, K0T * WPC * 128], BF16)

                with tc.tile_pool(name="pp", bufs=1) as pp, \
                     tc.tile_pool(name="psP", bufs=1, space="PSUM") as psP:
                    ncol = [(j * 512, min(512, IN - j * 512))
                            for j in range((IN + 511) // 512)]
                    zmasks, rowabs = [], []
                    cs_sb = pp.tile([1, IN], F32, tag="cs_sb")
                    cpss = [psP.tile([1, 512], F32, tag=f"cs{j}", name=f"cs{j}",
                                     space="PSUM") for j in range(len(ncol))]
                    with nc.named_scope("P1"):
                        for m in range(WPC):
                            pr = _wrows(m)
                            ft = pp.tile([128, IN], F32, tag="ftA",
                                         name="ftA", bufs=3)
                            nc.sync.dma_start(out=ft[:pr, :],
                                              in_=feat[m * 128:m * 128 + pr, :])
                            for j, (c0, cw) in enumerate(ncol):
                                nc.tensor.matmul(out=cpss[j][:1, :cw],
                                                 lhsT=ones128[:pr, :],
                                                 rhs=ft[:pr, c0:c0 + cw],
                                                 start=(m == 0),
                                                 stop=(m == WPC - 1))
                            zm = pp.tile([128, IN], BF16, tag=f"zm{m}",
                                         name=f"zm{m}")
                            nc.vector.tensor_scalar(out=zm[:pr, :],
                                                    in0=ft[:pr, :],
                                                    scalar1=0.0, scalar2=None,
                                                    op0=OP.is_equal)
                            zmasks.append(zm)
                            ra = pp.tile([128, 1], F32, tag=f"ra{m}",
                                         name=f"ra{m}")
                            nc.vector.tensor_reduce(out=ra[:pr, :],
                                                    in_=ft[:pr, :],
                                                    axis=AX.X, op=OP.add,
                                                    apply_absolute_value=True)
                            rowabs.append(ra)
                        for j, (c0, cw) in enumerate(ncol):
                            nc.scalar.copy(out=cs_sb[:, c0:c0 + cw],
                                           in_=cpss[j][:1, :cw])
                        nc.gpsimd.dma_start(out=cs_bounce[:], in_=cs_sb[:])
                        nc.gpsimd.collective_compute(
                            "AllReduce", OP.add,
                            replica_groups=[list(range(NC))],
                            ins=[cs_bounce[:]], outs=[cs_sum[:]])

                    with nc.named_scope("P2"):
                        meanh = pp.tile([1, IN], F32, tag="meanh")
                        nc.sync.dma_start(out=meanh[:], in_=cs_sum[:])
                        nc.scalar.mul(out=meanh[:], in_=meanh[:], mul=0.5 / N)
                        meanb = pp.tile([128, IN], F32, tag="meanb")
                        for j, (c0, cw) in enumerate(ncol):
                            bps = psP.tile([128, 512], F32, tag="bps",
                                           name="bps", space="PSUM")
                            nc.tensor.matmul(out=bps[:, :cw],
                                             lhsT=ones_row[:, :],
                                             rhs=meanh[:, c0:c0 + cw],
                                             start=True, stop=True)
                            nc.scalar.copy(out=meanb[:, c0:c0 + cw],
                                           in_=bps[:, :cw])
                        for m in range(WPC):
                            pr = _wrows(m)
                            zm, ra = zmasks[m], rowabs[m]
                            ft = pp.tile([128, KA], F32, tag="ftB",
                                         name="ftB", bufs=3)
                            nc.sync.dma_start(out=ft[:pr, 0:IN],
                                              in_=feat[m * 128:m * 128 + pr, :])
                            nc.vector.memset(ft[:, IN:KA], 1.0)
                            zmf = pp.tile([128, IN], F32, tag="zmf", name="zmf",
                                          bufs=2)
                            nc.vector.tensor_tensor(out=zmf[:pr, :],
                                                    in0=zm[:pr, :],
                                                    in1=meanb[:pr, :],
                                                    op=OP.mult)
                            nc.vector.tensor_tensor(out=ft[:pr, 0:IN],
                                                    in0=ft[:pr, 0:IN],
                                                    in1=zmf[:pr, :], op=OP.add)
                            bsum = work.tile([128, 1], F32, tag="bsum")
                            nc.vector.tensor_reduce(out=bsum[:pr, :],
                                                    in_=zmf[:pr, :],
                                                    axis=AX.X, op=OP.add,
                                                    apply_absolute_value=True)
                            nc.vector.tensor_tensor(out=bsum[:pr, :],
                                                    in0=bsum[:pr, :],
                                                    in1=ra[:pr, :], op=OP.add)
                            nc.vector.tensor_scalar(out=bsum[:pr, :],
                                                    in0=bsum[:pr, :],
                                                    scalar1=1e-12, scalar2=None,
                                                    op0=OP.max)
                            rinv = work.tile([128, 1], F32, tag="rinv")
                            nc.vector.reciprocal(out=rinv[:pr, :],
                                                 in_=bsum[:pr, :])
                            nc.vector.tensor_scalar(out=ft[:pr, 0:IN],
                                                    in0=ft[:pr, 0:IN],
                                                    scalar1=rinv[:pr, 0:1],
                                                    scalar2=None, op0=OP.mult)
                            for k in range(K0T):
                                kw = min(128, KA - k * 128)
                                tps = psP.tile([128, 128], F32, tag="tps",
                                               name="tps", space="PSUM",
                                               bufs=3)
                                nc.tensor.transpose(
                                    out=tps[:kw, :pr],
                                    in_=ft[:pr, k * 128:k * 128 + kw],
                                    identity=ident[:pr, :pr])
                                nc.scalar.copy(
                                    out=hT[:kw, (k * WPC + m) * 128:
                                           (k * WPC + m) * 128 + pr],
                                    in_=tps[:kw, :pr])

                # ---------- D0 dense ----------
                def dense0(dpool, psD, wdram, wtag, width, chunks, emit,
                           psbufs=2):
                    wkt = [dpool.tile([128, width], BF16, tag=f"{wtag}{k}",
                                      name=f"{wtag}{k}", bufs=2)
                           for k in range(K0T)]
                    for k in range(K0T):
                        kw = min(128, KA - k * 128)
                        nc.sync.dma_start(out=wkt[k][:kw, :],
                                          in_=wdram[k * 128:k * 128 + kw, :])
                    for m in range(WPC):
                        pr = _wrows(m)
                        ops = psD.tile([128, width], F32, tag=f"ps{wtag}",
                                       name=f"ps{wtag}", space="PSUM",
                                       bufs=psbufs)
                        for k in range(K0T):
                            kw = min(128, KA - k * 128)
                            for c0, cw in chunks:
                                nc.tensor.matmul(
                                    out=ops[:pr, c0:c0 + cw],
                                    lhsT=hT[:kw, (k * WPC + m) * 128:
                                            (k * WPC + m) * 128 + pr],
                                    rhs=wkt[k][:kw, c0:c0 + cw],
                                    start=(k == 0), stop=(k == K0T - 1))
                        emit(m, pr, ops)

                with nc.named_scope("D0lz"), \
                     tc.tile_pool(name="dpoolA", bufs=1) as dpA, \
                     tc.tile_pool(name="psDA", bufs=2, space="PSUM") as psDA:
                    osbs = {}

                    def emit_l(m, pr, ops):
                        osb = dpA.tile([128, 1064], BF16, tag="osb",
                                       name="osb", bufs=2)
                        nc.scalar.copy(out=osb[:pr, 0:1028],
                                       in_=ops[:pr, :])
                        osbs[m] = osb

                    def emit_z(m, pr, ops):
                        osb = osbs.pop(m)
                        nc.scalar.copy(out=osb[:pr, 1028:1064], in_=ops[:pr, :])
                        nc.sync.dma_start(
                            out=tab0_c[m * 128:m * 128 + pr, 0:1064],
                            in_=osb[:pr, :])

                    dense0(dpA, psDA, wl0a, "wl", 1028,
                           [(0, 512), (512, 512), (1024, 4)], emit_l)
                    dense0(dpA, psDA, w2a, "wz", 36, [(0, 36)], emit_z)
                    nc.gpsimd.collective_compute(
                        "AllGather", OP.bypass,
                        replica_groups=[list(range(NC))],
                        ins=[tab0_c[:]], outs=[tab0_full[:]])

                with nc.named_scope("D0r"), \
                     tc.tile_pool(name="dpoolB", bufs=1) as dpB, \
                     tc.tile_pool(name="psDB", bufs=1, space="PSUM") as psDB:

                    def emit_r(m, pr, ops):
                        nc.vector.tensor_copy(
                            out=fdw_all[:pr, m * FDW:(m + 1) * FDW],
                            in_=ops[:pr, :])

                    def emit_res(m, pr, ops):
                        nc.vector.tensor_copy(
                            out=res_all[:pr, m * HD0:(m + 1) * HD0],
                            in_=ops[:pr, :])

                    dense0(dpB, psDB, wr0a, "wr", FDW,
                           [(0, 512), (512, 512), (1024, 8)], emit_r,
                           psbufs=2)
                    dense0(dpB, psDB, wres0a, "wq", HD0,
                           [(0, 512), (512, 512)], emit_res, psbufs=1)

            # ============ Phase E0 + fused D1 (h1T alive) ============
            with tc.tile_pool(name="h1pool", bufs=1) as h1pool, \
                 tc.tile_pool(name="d1pool", bufs=1) as d1pool:
                h1T = h1pool.tile([128, K1T * WPC * 128], BF16)
                wk1 = [d1pool.tile([128, 104], BF16, tag=f"wk1{k}",
                                   name=f"wk1{k}") for k in range(K1T)]
                for k in range(K1T):
                    nc.sync.dma_start(out=wk1[k][:, :],
                                      in_=wcat1[k * 128:(k + 1) * 128, :])

                with nc.named_scope("E0"), \
                     tc.tile_pool(name="e0pool", bufs=2) as e0p, \
                     tc.tile_pool(name="psE0", bufs=1, space="PSUM") as psE0:
                    for w in range(WPC):
                        T = Ts[w]
                        nloc = _wrows(w)
                        co = toff[w]
                        fsg = e0p.tile([128, Tmax * TW], BF16, tag="fsg",
                                       bufs=3)
                        fsg_r = fsg[:].rearrange("p (t e) -> p t e", t=Tmax)
                        th = (T + 1) // 2
                        tokens = [gather_async(fsg_r[:, 0:th, :],
                                               tab0_full[:], co, th, TW,
                                               (2 * w) % NQ)]
                        if T > th:
                            tokens.append(gather_async(
                                fsg_r[:, th:T, :], tab0_full[:], co + th,
                                T - th, TW, (2 * w + 1) % NQ))
                        selw = e0p.tile([128, Tmax * 128], BF16, tag="selw")
                        nc.sync.dma_start(out=selw[:, :T * 128],
                                          in_=selb[:, co * 128:
                                                   (co + T) * 128])
                        selTw = e0p.tile([128, Tmax * 128], BF16, tag="selTw")
                        nc.sync.dma_start(out=selTw[:, :T * 128],
                                          in_=selTb[:, co * 128:
                                                    (co + T) * 128])
                        fdm = fdw_all[:, w * FDW:(w + 1) * FDW]

                        o_ps = psE0.tile([128, HD0], F32, tag="o_ps",
                                         name="o_ps", space="PSUM")
                        # ozm regions: [0:40] oz agg | [64:72],[72:80] ext A/B
                        # | [128:232] fused-D1 accumulator
                        ozm = psE0.tile([128, 512], F32, tag="ozm",
                                        name="ozm", space="PSUM")
                        # bf16 PSUM pair for PE transposes (h1T build)
                        tqt = psE0.tile([128, 256], BF16, tag="tqt",
                                        name="tqt", space="PSUM")
                        gather_wait(tokens, [nc.tensor, nc.vector])
                        for t in range(T):
                            st = selTw[:, t * 128:(t + 1) * 128]
                            ss = selw[:, t * 128:(t + 1) * 128]
                            ft_ = fsg[:, t * TW:t * TW + 1032]
                            vb = vbs[:, co + t:co + t + 1]
                            s_ps = psE0.tile([128, HD0], F32, tag="s_ps",
                                             name="s_ps", space="PSUM",
                                             bufs=2)
                            eo = 64 + 8 * (t % 2)
                            ext = ozm[:, eo:eo + 8]
                            for c0, cw in ((0, 512), (512, 512)):
                                nc.tensor.matmul(out=s_ps[:, c0:c0 + cw],
                                                 lhsT=st[:nloc, :],
                                                 rhs=fdm[:nloc, c0:c0 + cw],
                                                 start=True, stop=False)
                            nc.tensor.matmul(out=ext,
                                             lhsT=st[:nloc, :],
                                             rhs=fdm[:nloc, 1024:1032],
                                             start=True, stop=False)
                            for c0, cw in ((0, 512), (512, 512)):
                                nc.tensor.matmul(out=s_ps[:, c0:c0 + cw],
                                                 lhsT=ident16[:, :],
                                                 rhs=ft_[:, c0:c0 + cw],
                                                 start=False, stop=True)
                            nc.tensor.matmul(out=ext,
                                             lhsT=ident16[:, :],
                                             rhs=ft_[:, 1024:1032],
                                             start=False, stop=True)
                            u = e0p.tile([128, HD0], BF16, tag="u")
                            nc.scalar.activation(out=u[:], in_=s_ps[:, 0:1024],
                                                 func=AF.Abs)
                            nc.vector.tensor_tensor(out=u[:], in0=u[:],
                                                    in1=a0b[:], op=OP.mult)
                            lgab = work.tile([128, H], F32, tag="lgab")
                            nc.vector.tensor_reduce(
                                out=lgab[:],
                                in_=u[:].rearrange("p (h d) -> p h d", h=H),
                                axis=AX.X, op=OP.add)
                            # lg8 = [lrelu(ed) 0:4 | a.lrelu(s) 4:8]
                            lg8 = work.tile([128, 8], F32, tag="lg8")
                            lr = work.tile([128, H], F32, tag="lr")
                            nc.vector.tensor_scalar(out=lr[:], in0=ext[:, 4:8],
                                                    scalar1=0.2, scalar2=None,
                                                    op0=OP.mult)
                            nc.vector.tensor_tensor(out=lg8[:, 0:4],
                                                    in0=ext[:, 4:8],
                                                    in1=lr[:], op=OP.max)
                            nc.vector.tensor_tensor(out=lg8[:, 4:8],
                                                    in0=lgab[:],
                                                    in1=ext[:, 0:4],
                                                    op=OP.add)
                            # elf8 = [zexp 0:4 | elf 4:8]
                            elf8 = work.tile([128, 8], F32, tag="elf8")
                            nc.scalar.activation(out=elf8[:], in_=lg8[:],
                                                 func=AF.Exp, bias=vb)
                            fv = e0p.tile([128, HD0], BF16, tag="fv")
                            for h in range(H):
                                if h < 2:
                                    nc.scalar.mul(
                                        out=fv[:, h * D0:(h + 1) * D0],
                                        in_=fsg[:, t * TW + h * D0:
                                                t * TW + (h + 1) * D0],
                                        mul=elf8[:, 4 + h:5 + h])
                                else:
                                    nc.vector.tensor_scalar(
                                        out=fv[:, h * D0:(h + 1) * D0],
                                        in0=fsg[:, t * TW + h * D0:
                                                t * TW + (h + 1) * D0],
                                        scalar1=elf8[:, 4 + h:5 + h],
                                        scalar2=None, op0=OP.mult)
                            zv = e0p.tile([128, 40], BF16, tag="zv")
                            nc.vector.tensor_copy(out=zv[:, 32:40],
                                                  in_=elf8[:])
                            nc.vector.tensor_tensor(
                                out=zv[:, 0:32].rearrange(
                                    "p (h d) -> p h d", h=H),
                                in0=fsg[:, t * TW + 1032:t * TW + 1064]
                                .rearrange("p (h d) -> p h d", h=H),
                                in1=zv[:, 32:36].to_broadcast([128, H, D1]),
                                op=OP.mult)
                            for c0, cw in ((0, 512), (512, 512)):
                                nc.tensor.matmul(out=o_ps[:nloc, c0:c0 + cw],
                                                 lhsT=ss[:, :nloc],
                                                 rhs=fv[:, c0:c0 + cw],
                                                 start=(t == 0),
                                                 stop=(t == T - 1))
                            nc.tensor.matmul(out=ozm[:nloc, 0:40],
                                             lhsT=ss[:, :nloc], rhs=zv[:, :],
                                             start=(t == 0), stop=(t == T - 1))

                        # epilogue: normalize, residual, relu, transpose
                        idn = work.tile([128, H], F32, tag="idn")
                        nc.vector.tensor_scalar(out=idn[:nloc, :],
                                                in0=ozm[:nloc, 36:40],
                                                scalar1=1e-9, scalar2=None,
                                                op0=OP.max)
                        nc.vector.reciprocal(out=idn[:nloc, :],
                                             in_=idn[:nloc, :])
                        idn2 = work.tile([128, H], F32, tag="idn2")
                        nc.vector.tensor_scalar(out=idn2[:nloc, :],
                                                in0=ozm[:nloc, 32:36],
                                                scalar1=1e-9, scalar2=None,
                                                op0=OP.max)
                        nc.vector.reciprocal(out=idn2[:nloc, :],
                                             in_=idn2[:nloc, :])
                        ho = e0p.tile([128, HD0], BF16, tag="ho")
                        for h in range(H):
                            nc.vector.tensor_scalar(
                                out=ho[:nloc, h * D0:(h + 1) * D0],
                                in0=o_ps[:nloc, h * D0:(h + 1) * D0],
                                scalar1=idn[:nloc, h:h + 1],
                                scalar2=None, op0=OP.mult)
                        nc.vector.tensor_tensor(
                            out=ho[:nloc, :], in0=ho[:nloc, :],
                            in1=res_all[:nloc, w * HD0:(w + 1) * HD0],
                            op=OP.add)
                        nc.scalar.activation(out=ho[:nloc, :],
                                             in_=ho[:nloc, :], func=AF.Relu)
                        for k in range(K1T):
                            to = 128 * (k % 2)
                            tq = tqt[:, to:to + 128]
                            nc.tensor.transpose(
                                out=tq,
                                in_=ho[:, k * 128:(k + 1) * 128],
                                identity=ident16[:, :])
                            dst = h1T[:, (k * WPC + w) * 128:
                                      (k * WPC + w + 1) * 128]
                            if k % 2:
                                nc.scalar.copy(out=dst, in_=tq)
                            else:
                                nc.vector.tensor_copy(out=dst, in_=tq)
                        nc.vector.tensor_tensor(
                            out=h3_all[:nloc, w * HD1:(w + 1) * HD1]
                            .rearrange("p (h d) -> p h d", h=H),
                            in0=ozm[:nloc, 0:32].rearrange(
                                "p (h d) -> p h d", h=H),
                            in1=idn2[:nloc, :].to_broadcast([nloc, H, D1]),
                            op=OP.mult)

                        # fused D1 for this window (accum in ozm[128:232])
                        for k in range(K1T):
                            nc.tensor.matmul(
                                out=ozm[:nloc, 128:232],
                                lhsT=h1T[:, (k * WPC + w) * 128:
                                         (k * WPC + w) * 128 + nloc],
                                rhs=wk1[k][:, :],
                                start=(k == 0), stop=(k == K1T - 1))
                        osb1 = e0p.tile([128, 36], BF16, tag="osb1")
                        nc.scalar.copy(out=osb1[:nloc, 0:32],
                                       in_=ozm[:nloc, 128:160])
                        nc.scalar.copy(out=osb1[:nloc, 32:36],
                                       in_=ozm[:nloc, 224:228])
                        nc.sync.dma_start(
                            out=tab1_c[w * 128:w * 128 + nloc, 0:36],
                            in_=osb1[:nloc, :])
                        nc.scalar.copy(
                            out=fd1_all[:nloc, w * FD1W:w * FD1W + 32],
                            in_=ozm[:nloc, 160:192])
                        nc.scalar.copy(
                            out=fd1_all[:nloc, w * FD1W + 32:(w + 1) * FD1W],
                            in_=ozm[:nloc, 228:232])
                        nc.scalar.copy(out=res1_t[w][:nloc, :],
                                       in_=ozm[:nloc, 192:224])
                    nc.gpsimd.collective_compute(
                        "AllGather", OP.bypass,
                        replica_groups=[list(range(NC))],
                        ins=[tab1_c[:]], outs=[tab1_full[:]])

            # ============ Phase E1 ============
            with nc.named_scope("E1"), \
                 tc.tile_pool(name="e1pool", bufs=2) as e1p, \
                 tc.tile_pool(name="psE1", bufs=1, space="PSUM") as psE1:
                f1ga = e1p.tile([128, totT * TW1], BF16, tag="f1ga", bufs=1)
                f1gr = f1ga[:].rearrange("p (t e) -> p t e", t=totT)
                ptok = {}
                for w0 in range(WPC):
                    t0 = toff[w0]
                    t1 = t0 + Ts[w0]
                    tok = gather_async(f1gr[:, t0:t1, :], tab1_full[:],
                                       t0, t1 - t0, TW1, w0 % NQ)
                    ptok[w0] = [tok]
                for w in range(WPC):
                    T = Ts[w]
                    nloc = _wrows(w)
                    co = toff[w]
                    tokens1 = ptok[w]
                    selw = e1p.tile([128, Tmax * 128], BF16, tag="selw")
                    nc.sync.dma_start(out=selw[:, :T * 128],
                                      in_=selb[:, co * 128:(co + T) * 128])
                    selTw = e1p.tile([128, Tmax * 128], BF16, tag="selTw")
                    nc.sync.dma_start(out=selTw[:, :T * 128],
                                      in_=selTb[:, co * 128:(co + T) * 128])
                    fdm1 = fd1_all[:, w * FD1W:(w + 1) * FD1W]

                    o1_ps = psE1.tile([128, 36], F32, tag="o1", name="o1",
                                      space="PSUM")
                    s1a = psE1.tile([128, Tmax * 32], F32, tag="s1a",
                                    name="s1a", space="PSUM")
                    ex1 = psE1.tile([128, Tmax * 4], F32, tag="ex1",
                                    name="ex1", space="PSUM")
                    gather_wait(tokens1, [nc.tensor, nc.vector])
                    for t in range(T):
                        st = selTw[:, t * 128:(t + 1) * 128]
                        nc.tensor.matmul(out=s1a[:, t * 32:(t + 1) * 32],
                                         lhsT=st[:nloc, :],
                                         rhs=fdm1[:nloc, 0:32],
                                         start=True, stop=False)
                        nc.tensor.matmul(out=ex1[:, t * 4:(t + 1) * 4],
                                         lhsT=st[:nloc, :],
                                         rhs=fdm1[:nloc, 32:36],
                                         start=True, stop=False)
                        nc.tensor.matmul(out=s1a[:, t * 32:(t + 1) * 32],
                                         lhsT=ident16[:, :],
                                         rhs=f1ga[:, (co + t) * TW1:
                                                  (co + t) * TW1 + 32],
                                         start=False, stop=True)
                        nc.tensor.matmul(out=ex1[:, t * 4:(t + 1) * 4],
                                         lhsT=ident16[:, :],
                                         rhs=f1ga[:, (co + t) * TW1 + 32:
                                                  (co + t) * TW1 + 36],
                                         start=False, stop=True)
                    u1a = e1p.tile([128, Tmax * 32], BF16, tag="u1a")
                    nc.scalar.activation(out=u1a[:, :T * 32],
                                         in_=s1a[:, :T * 32], func=AF.Abs)
                    nc.vector.tensor_tensor(out=u1a[:, :T * 32],
                                            in0=u1a[:, :T * 32],
                                            in1=a1bT[:, :T * 32], op=OP.mult)
                    lgab1 = e1p.tile([128, Tmax * 4], F32, tag="lgab1")
                    nc.vector.tensor_reduce(
                        out=lgab1[:, :T * 4],
                        in_=u1a[:, :T * 32].rearrange("p (q d) -> p q d",
                                                      d=D1),
                        axis=AX.X, op=OP.add)
                    nc.vector.tensor_tensor(out=lgab1[:, :T * 4],
                                            in0=lgab1[:, :T * 4],
                                            in1=ex1[:, :T * 4], op=OP.add)
                    nc.vector.tensor_tensor(out=lgab1[:, :T * 4],
                                            in0=lgab1[:, :T * 4],
                                            in1=vb4[:, co * 4:(co + T) * 4],
                                            op=OP.add)
                    el1a = e1p.tile([128, Tmax * 4], F32, tag="el1a")
                    nc.scalar.activation(out=el1a[:, :T * 4],
                                         in_=lgab1[:, :T * 4], func=AF.Exp)
                    fv1a = e1p.tile([128, Tmax * 36], BF16, tag="fv1a")
                    nc.vector.tensor_copy(
                        out=fv1a[:].rearrange("p (t e) -> p t e",
                                              t=Tmax)[:, :T, 32:36],
                        in_=el1a[:, :T * 4].rearrange("p (t h) -> p t h",
                                                      h=H))
                    el1r = el1a[:, :T * 4].rearrange("p (t h) -> p t h", h=H)
                    f1r = f1gr[:, co:co + T, :]
                    fv1r = fv1a[:].rearrange("p (t e) -> p t e", t=Tmax)
                    for h in range(H):
                        nc.vector.tensor_tensor(
                            out=fv1r[:, :T, h * D1:(h + 1) * D1],
                            in0=f1r[:, :T, h * D1:(h + 1) * D1],
                            in1=el1r[:, :, h:h + 1].to_broadcast(
                                [128, T, D1]),
                            op=OP.mult)
                    for t in range(T):
                        ss = selw[:, t * 128:(t + 1) * 128]
                        nc.tensor.matmul(out=o1_ps[:nloc, :],
                                         lhsT=ss[:, :nloc],
                                         rhs=fv1a[:, t * 36:(t + 1) * 36],
                                         start=(t == 0), stop=(t == T - 1))
                    idn1 = work.tile([128, H], F32, tag="idn1")
                    nc.vector.tensor_scalar(out=idn1[:nloc, :],
                                            in0=o1_ps[:nloc, 32:36],
                                            scalar1=1e-9, scalar2=None,
                                            op0=OP.max)
                    nc.vector.reciprocal(out=idn1[:nloc, :],
                                         in_=idn1[:nloc, :])
                    oo = work.tile([128, HD1], BF16, tag="oo")
                    nc.vector.tensor_tensor(
                        out=oo[:nloc, :].rearrange("p (h d) -> p h d", h=H),
                        in0=o1_ps[:nloc, 0:32].rearrange("p (h d) -> p h d",
                                                         h=H),
                        in1=idn1[:nloc, :].to_broadcast([nloc, H, D1]),
                        op=OP.mult)
                    nc.vector.tensor_tensor(out=oo[:nloc, :], in0=oo[:nloc, :],
                                            in1=res1_t[w][:nloc, :],
                                            op=OP.add)
                    nc.vector.tensor_tensor(out=oo[:nloc, :], in0=oo[:nloc, :],
                                            in1=b1b[:nloc, :], op=OP.add)
                    hhw = work.tile([128, HD1], BF16, tag="hhw")
                    nc.scalar.activation(out=hhw[:nloc, :],
                                         in_=oo[:nloc, :], func=AF.Relu)
                    # fused final dense: out = [h3 | hh] @ Wlin + blin
                    tpf = psE1.tile([128, 256], BF16, tag="tpf",
                                    name="tpf", space="PSUM")
                    nc.tensor.transpose(
                        out=tpf[:HD1, 0:nloc],
                        in_=h3_all[:nloc, w * HD1:(w + 1) * HD1],
                        identity=ident16[:nloc, :nloc])
                    nc.tensor.transpose(
                        out=tpf[:HD1, 128:128 + nloc],
                        in_=hhw[:nloc, :],
                        identity=ident16[:nloc, :nloc])
                    catm = work.tile([64, 128], BF16, tag="catm")
                    nc.scalar.copy(out=catm[0:HD1, :nloc],
                                   in_=tpf[:HD1, 0:nloc])
                    nc.scalar.copy(out=catm[HD1:2 * HD1, :nloc],
                                   in_=tpf[:HD1, 128:128 + nloc])
                    fp = psE1.tile([128, OUT], F32, tag="fin", name="fin",
                                   space="PSUM")
                    nc.tensor.matmul(out=fp[:nloc, :],
                                     lhsT=catm[:, :nloc],
                                     rhs=wlsb[:], start=True, stop=True)
                    osbf = work.tile([128, OUT], F32, tag="osbF")
                    nc.vector.tensor_tensor(out=osbf[:nloc, :],
                                            in0=fp[:nloc, :],
                                            in1=blinb[:nloc, :], op=OP.add)
                    nc.sync.dma_start(out=out_ext[w * 128:w * 128 + nloc, :],
                                      in_=osbf[:nloc, :])

    nc.compile()
    return nc


def _prep_edges(src, dst):
    order = np.argsort(dst, kind="stable")
    ss = src[order].astype(np.int64)
    ds = dst[order].astype(np.int64)
    cnt = np.zeros((NC, WPC), np.int64)
    bounds = {}
    for c in range(NC):
        for w in range(WPC):
            lo = c * NPC + w * 128
            hi = min(c * NPC + (w + 1) * 128, (c + 1) * NPC)
            e0 = np.searchsorted(ds, lo, side="left")
            e1 = np.searchsorted(ds, hi, side="left")
            cnt[c, w] = e1 - e0
            bounds[(c, w)] = (e0, e1)
    nws = [int(cnt[:, w].max()) for w in range(WPC)]
    Ts = [max(1, math.ceil(nv / 128)) for nv in nws]
    totT = sum(Ts)
    per_core = []
    for c in range(NC):
        sidx = np.zeros((128, 8 * totT), np.int16)
        vb = np.full((128, totT), NEG, np.float32)
        selm = np.zeros((128, totT * 128), NPBF)
        selTm = np.zeros((128, totT * 128), NPBF)
        co = 0
        for w in range(WPC):
            T = Ts[w]
            e0, e1 = bounds[(c, w)]
            k = e1 - e0
            slots = T * 128
            s = np.zeros(slots, np.int16)
            d = np.zeros(slots, np.int64)
            v = np.full(slots, NEG, np.float32)
            s[:k] = ss[e0:e1]
            d[:k] = ds[e0:e1] - (c * NPC + w * 128)
            v[:k] = 0.0
            cols = s.reshape(8 * T, 16).T
            sidx[:, 8 * co:8 * (co + T)] = np.tile(cols, (8, 1))
            vb[:, co:co + T] = v.reshape(T, 128).T
            for t in range(T):
                M = np.zeros((128, 128), np.float32)
                dt_ = d[t * 128:(t + 1) * 128]
                nvalid = max(0, min(128, k - t * 128))
                if nvalid:
                    M[np.arange(nvalid), dt_[:nvalid]] = 1.0
                selm[:, (co + t) * 128:(co + t + 1) * 128] = M.astype(NPBF)
                selTm[:, (co + t) * 128:(co + t + 1) * 128] = \
                    M.T.astype(NPBF)
            co += T
        per_core.append((sidx, vb, selm, selTm))
    return Ts, per_core


def _bd(a, scale):
    # block-diagonal [H*Dh, H] from a [H, Dh]
    h, dh = a.shape
    out = np.zeros((h * dh, h), np.float32)
    for i in range(h):
        out[i * dh:(i + 1) * dh, i] = scale * a[i]
    return out


def kernel(features, src, dst, textMask, audioMask, videoMask, W2, a2,
           Wl0, Wr0, a0, Wres0, b0, Wl1, Wr1, a1, Wres1, b1, Wlin, blin):
    features = np.asarray(features, np.float32)
    src = np.asarray(src, np.int32)
    dst = np.asarray(dst, np.int32)

    Ts, per_core = _prep_edges(src, dst)
    Tmax = max(Ts)
    key = tuple(Ts)
    if key not in _compiled:
        _compiled.clear()
        _compiled[key] = _build_program(Ts)
    nc = _compiled[key]

    maskSum = (np.asarray(textMask) + np.asarray(audioMask)
               + np.asarray(videoMask)).astype(np.float32)

    def aug(Wm, brow=None):
        o = np.zeros((KA, Wm.shape[1]), np.float32)
        o[:IN] = Wm * maskSum[:, None]
        if brow is not None:
            o[IN] = brow
        return o

    a0f = np.asarray(a0, np.float32)
    a1f = np.asarray(a1, np.float32)
    a2f = np.asarray(a2, np.float32)
    wl0f = aug(np.asarray(Wl0, np.float32))
    wr0f = aug(np.asarray(Wr0, np.float32))
    w2flat = np.asarray(W2, np.float32).transpose(1, 0, 2).reshape(IN, HD1)
    w2f = aug(w2flat)
    wl1f = np.asarray(Wl1, np.float32)
    wr1f = np.asarray(Wr1, np.float32)

    wl0aug = np.concatenate([wl0f, wl0f @ _bd(a0f, 0.6)], axis=1)
    wr0aug = np.concatenate([wr0f, wr0f @ _bd(a0f, 0.6),
                             w2f @ _bd(a2f[:, D1:], 1.0)], axis=1)
    # order [es | z] so tab0 cols land as [.. | es 1028:1032 | z 1032:1064]
    w2aug = np.concatenate([w2f @ _bd(a2f[:, :D1], 1.0), w2f], axis=1)
    wcat1 = np.concatenate([wl1f, wr1f, np.asarray(Wres1, np.float32),
                            wl1f @ _bd(a1f, 0.6), wr1f @ _bd(a1f, 0.6)],
                           axis=1)

    shared = {
        "wl0a": wl0aug.astype(NPBF),
        "wr0a": wr0aug.astype(NPBF),
        "wres0a": aug(np.asarray(Wres0, np.float32),
                      np.asarray(b0, np.float32)).astype(NPBF),
        "w2a": w2aug.astype(NPBF),
        "wcat1": wcat1.astype(NPBF),
        "wlin": np.asarray(Wlin, np.float32).astype(NPBF),
        "a0b04": np.tile((0.4 * a0f).reshape(1, HD0), (128, 1)).astype(NPBF),
        "a1b04": np.tile((0.4 * a1f).reshape(1, HD1), (128, 1)),
        "b1bc": np.tile(np.asarray(b1, np.float32).reshape(1, HD1), (128, 1)),
        "a1bt": np.tile((0.4 * a1f).reshape(1, HD1),
                        (128, Tmax)).astype(NPBF),
        "blinbc": np.tile(np.asarray(blin, np.float32).reshape(1, OUT),
                          (128, 1)),
    }
    in_maps = []
    for c in range(NC):
        sidx, vb, selm, selTm = per_core[c]
        m = dict(shared)
        m["feat"] = np.ascontiguousarray(features[c * NPC:(c + 1) * NPC])
        m["srcidx"] = sidx
        m["vbias"] = vb
        m["vbias4"] = np.repeat(vb, 4, axis=1)
        m["selb"] = selm
        m["selTb"] = selTm
        in_maps.append(m)

    res = run_bass_kernel_spmd(nc, in_maps, list(range(NC)))
    global last_exec_ns, last_result
    last_result = res
    last_exec_ns = getattr(res, "exec_time_ns", None)
    return np.concatenate(
        [np.asarray(res.results[c]["out"]) for c in range(NC)], axis=0)

